# revision 1
# baseline (speedup 1.0000x reference)
"""CrossWinAttention Trainium2 Bass kernel.

Problem (hardcoded shapes): q/k/v (2,6,8,8,8,8,128) f32, windowed attention
over l=x*y=64 windows per batch, each window has T = n*w1*w2 = 384 tokens of
dim 128; LN -> QKV proj -> 4-head attention (dhead 32) -> out proj -> mean
over n agents -> + skip.

Sharding: the 2*64 = 128 (b, l) windows are fully independent -> 16 windows
per NeuronCore across 8 cores (SPMD: same program, per-core data).

The kernel is latency/sync-bound, not compute-bound, so the design minimizes
instruction count and keeps every engine's strict-FIFO queue free of
head-of-line stalls:
  Phase 1 (per window): f32 window loads spread over 4 DMA queues; LN stats
    as square (DVE mul) + grouped tensor_reduce over the innermost axis
    (one instruction covers all 9 [128,128] tiles of a window).
  Batch point: ONE Ln + ONE Exp over all windows' variances
    (rstd_raw = 1/sqrt(128*(var+eps)); the sqrt(128) is folded into the
    host-side projection weights so no extra device scaling is needed).
    ln/exp share the softmax exp's ACT table set -> no table thrash.
  Phase 2 (per window, engines pipeline across windows):
  - normalize with tensor_scalar (mu, rstd_raw scalars) -> bf16
  - transpose 128x128 tiles on PE -> x^T; QKV projections on PE. LN affine,
    softmax scale and head_gate are folded into the weights on the host.
    q/k projected to [hd, t]; v to [t, hd]. All through one 2-buf PSUM pool.
  - dot: 4 heads row-packed (tile_position rows, contraction dh=32) into ONE
    [128, 4*384] 3-bank PSUM tile per kt; ONE Exp per kt straight out of
    PSUM -> bf16 (logits are tiny, no max subtraction needed)
  - denominators s[h, q] via ones-matmul over k partitions, col-packed with a
    32-wide replicated ones stationary so 1/s is a full [128, 384] DVE op
  - A^T[hd, q] = V^T-stationary x exp-moving, col-packed 4 heads -> the
    concatenated head layout needed by the output projection
  - out proj with wp/6, mean over n folded as 6 accumulating matmuls N=64
  - PE transpose [d,64] -> [64,d], add skip (DVE), store f32
"""

import os
from contextlib import ExitStack

import numpy as np
import ml_dtypes

import concourse.bass as bass
import concourse.tile as tile
from concourse import mybir
from concourse.bass_utils import run_bass_kernel_spmd
from concourse.masks import make_identity

# ---- problem constants (must match the grading reference) ----
B, NAG, X, Y, W1, W2 = 2, 6, 8, 8, 8, 8
DIM, HEADS, DHEAD = 128, 4, 32
HD = HEADS * DHEAD
EPS = 1e-5
SCALE = DHEAD ** -0.5
N_CORES = 8
L = X * Y                    # 64 windows per batch
NWIN = B * L                 # 128 windows total
WPC = NWIN // N_CORES        # 16 windows per core
T = NAG * W1 * W2            # 384 tokens per window
TT = T // 128                # 3 token tiles
WTOK = W1 * W2               # 64 output tokens per window
KK = 4                       # consecutive tokens per SBUF partition row
TP = T // KK                 # 96 partitions used by phase-1 tiles

F32 = mybir.dt.float32
BF16 = mybir.dt.bfloat16


def build_nc(n_win=WPC, qbias=False, kbias=False, norm_engine="vector", iters=1):
    """Build the per-core Bass module.

    iters > 1 replays the whole body iters times (Python unroll; For_i does
    not survive this walrus build) recomputing the same outputs; used only
    for wall-clock timing (amortizes host/RPC cost)."""
    nc = bass.Bass(trn_type="TRN2")

    qkvi = nc.dram_tensor("qkvin", [n_win, 3, T, DIM], F32, kind="ExternalInput")
    ski = nc.dram_tensor("skin", [n_win, WTOK, DIM], F32, kind="ExternalInput")
    wqd = nc.dram_tensor("wq", [DIM, HD], BF16, kind="ExternalInput")
    wkd = nc.dram_tensor("wk", [DIM, HD], BF16, kind="ExternalInput")
    wvd = nc.dram_tensor("wv", [DIM, HD], BF16, kind="ExternalInput")
    wpd = nc.dram_tensor("wp", [HD, DIM], BF16, kind="ExternalInput")
    bqd = nc.dram_tensor("bq", [1, HD], F32, kind="ExternalInput")
    bkd = nc.dram_tensor("bk", [1, HD], F32, kind="ExternalInput")
    outo = nc.dram_tensor("out", [n_win, WTOK, DIM], F32, kind="ExternalOutput")

    with tile.TileContext(nc) as tc, ExitStack() as ctx:
        consts = ctx.enter_context(tc.tile_pool(name="consts", bufs=1))
        pxin = ctx.enter_context(tc.tile_pool(name="pxin", bufs=1))
        psq = ctx.enter_context(tc.tile_pool(name="psq", bufs=2))
        pxn = ctx.enter_context(tc.tile_pool(name="pxn", bufs=1))
        pxt = ctx.enter_context(tc.tile_pool(name="pxt", bufs=2))
        pqkv = ctx.enter_context(tc.tile_pool(name="pqkv", bufs=3))
        pexp = ctx.enter_context(tc.tile_pool(name="pexp", bufs=2))
        prs = ctx.enter_context(tc.tile_pool(name="prs", bufs=2))
        pat = ctx.enter_context(tc.tile_pool(name="pat", bufs=2))
        ptail = ctx.enter_context(tc.tile_pool(name="ptail", bufs=2))
        pskip = ctx.enter_context(tc.tile_pool(name="pskip", bufs=6))
        # PSUM: 8 banks. pp (transp+proj) 2x1 + dot 1x2 + s/av 1x2 + z/zt 2
        pp = ctx.enter_context(tc.tile_pool(name="pp", bufs=2, space="PSUM"))
        psdot = ctx.enter_context(tc.tile_pool(name="psdot", bufs=1, space="PSUM"))
        pssav = ctx.enter_context(tc.tile_pool(name="pssav", bufs=1, space="PSUM"))
        psz = ctx.enter_context(tc.tile_pool(name="psz", bufs=1, space="PSUM"))

        # ---- constants ----
        wq_sb = consts.tile([DIM, HD], BF16, tag="wq")
        wk_sb = consts.tile([DIM, HD], BF16, tag="wk")
        wv_sb = consts.tile([DIM, HD], BF16, tag="wv")
        wp_sb = consts.tile([HD, DIM], BF16, tag="wp")
        nc.scalar.dma_start(out=wq_sb, in_=wqd[:, :])
        nc.scalar.dma_start(out=wk_sb, in_=wkd[:, :])
        nc.scalar.dma_start(out=wv_sb, in_=wvd[:, :])
        nc.scalar.dma_start(out=wp_sb, in_=wpd[:, :])
        bq_sb = consts.tile([1, HD], F32, tag="bq")
        bk_sb = consts.tile([1, HD], F32, tag="bk")
        if qbias:
            nc.scalar.dma_start(out=bq_sb, in_=bqd[:, :])
        if kbias:
            nc.scalar.dma_start(out=bk_sb, in_=bkd[:, :])
        ones32 = consts.tile([128, 32], BF16, tag="ones32")
        nc.vector.memset(ones32, 1.0)
        ones1 = consts.tile([1, T], BF16, tag="ones1")
        nc.vector.memset(ones1, 1.0)
        eps_t = consts.tile([TP, 1], F32, tag="eps")
        nc.vector.memset(eps_t, 128.0 * EPS)
        ident96 = consts.tile([TP, TP], BF16, tag="ident96")
        make_identity(nc, ident96[:, :])
        identf = consts.tile([128, 128], F32, tag="identf")
        make_identity(nc, identf[:, :])

        dmaq = [nc.scalar, nc.gpsimd]

        for _it in range(iters):
            # ---- phase 1: loads + LN stats ----
            # Each of the TP=96 partitions holds KK=4 consecutive tokens
            # (2KB contiguous DRAM rows -> ~4x DMA descriptor efficiency).
            # Token t lives at (p = t//4, k = t%4); agent n = p//16.
            sums = pxn.tile([TP, n_win, 3, KK], F32, tag="sums")
            ssq = pxn.tile([TP, n_win, 3, KK], F32, tag="ssq")
            x_w = []
            for w in range(n_win):
                xw = pxin.tile([TP, 3, KK, DIM], F32, tag=f"x{w}")
                x_w.append(xw)
                dmaq[w % 2].dma_start(
                    out=xw,
                    in_=qkvi[w].rearrange("i (p k) d -> p i k d", p=TP),
                )
                sq = psq.tile([TP, 3, KK, DIM], F32, tag="sq")
                nc.vector.tensor_mul(sq, xw, xw)
                nc.vector.tensor_reduce(
                    out=sums[:, w], in_=xw, axis=mybir.AxisListType.X,
                    op=mybir.AluOpType.add,
                )
                nc.vector.tensor_reduce(
                    out=ssq[:, w], in_=sq, axis=mybir.AxisListType.X,
                    op=mybir.AluOpType.add,
                )
            # ---- batch point: mu and rstd_raw = 1/sqrt(128*(var+eps)) ----
            # var*128 = ssq - mu*sums; sqrt(128) folded into host weights
            mu = pxn.tile([TP, n_win, 3, KK], F32, tag="mu")
            nc.vector.tensor_scalar(
                out=mu, in0=sums, scalar1=1.0 / DIM, scalar2=None,
                op0=mybir.AluOpType.mult,
            )
            var128 = pxn.tile([TP, n_win, 3, KK], F32, tag="var128")
            nc.vector.tensor_mul(var128, mu, sums)
            nc.vector.tensor_sub(var128, ssq, var128)
            lnv = pxn.tile([TP, n_win, 3, KK], F32, tag="lnv")
            nc.scalar.activation(
                out=lnv, in_=var128,
                func=mybir.ActivationFunctionType.Ln, bias=eps_t,
            )
            rstd = pxn.tile([TP, n_win, 3, KK], F32, tag="rstd")
            nc.scalar.activation(
                out=rstd, in_=lnv,
                func=mybir.ActivationFunctionType.Exp, scale=-0.5,
            )
            # ---- normalize all windows -> bf16 (DVE) ----
            xn_w = []
            for w in range(n_win):
                xn = pxn.tile([TP, 3, KK, DIM], BF16, tag=f"xn{w}")
                xn_w.append(xn)
                for i in range(3):
                    for k in range(KK):
                        nc.vector.tensor_scalar(
                            out=xn[:, i, k, :],
                            in0=x_w[w][:, i, k, :],
                            scalar1=mu[:, w, i, k : k + 1],
                            scalar2=rstd[:, w, i, k : k + 1],
                            op0=mybir.AluOpType.subtract,
                            op1=mybir.AluOpType.mult,
                        )

            # ---- phase 2: software-pipelined per-window attention ----
            # Stages per window, skewed so each engine's strict-FIFO queue
            # head is always ready (cross-engine handoffs cost ~2us each if
            # the consumer queue blocks on them):
            #   A(w): transposes + projections (PE) + evacs (DVE)
            #   C(w): dot + exp   (PE + ACT)     -- issued one window behind A
            #   D(w): s/av + 1/s  (PE + DVE)     -- two behind
            #   E(w): out proj + transpose + skip add + store -- three behind
            skip_w, qT_w, kT_w, vh_w, expT_w, aT_w = {}, {}, {}, {}, {}, {}

            def stage_a(w):
                # skip rows permuted to the device token order (k, r)
                skip_sb = pskip.tile([WTOK, DIM], F32, tag="skip")
                nc.scalar.dma_start(
                    out=skip_sb,
                    in_=ski[w].rearrange("(r k) d -> k r d", k=KK),
                )
                skip_w[w] = skip_sb
                xn = xn_w[w]
                # transpose to [d, t'] on PE (xn stationary, identity moving);
                # x^T column c = k*96 + p <-> token t = 4p + k
                xT_sb = pxt.tile([128, 3, T], BF16, tag="xT")
                for i in range(3):
                    tp = pp.tile([128, T], F32, tag="pj")
                    for k in range(KK):
                        nc.tensor.matmul(
                            tp[:, k * TP : (k + 1) * TP],
                            lhsT=xn[:, i, k, :], rhs=ident96[:, :],
                            start=True, stop=True,
                        )
                    nc.vector.tensor_copy(xT_sb[:, i, :], tp)
                # projections: q, k -> [hd, t]
                qT_sb = pqkv.tile([HD, T], BF16, tag="qT")
                kT_sb = pqkv.tile([HD, T], BF16, tag="kT")
                for i, (w_sb, b_sb, has_b, dst) in enumerate(
                    ((wq_sb, bq_sb, qbias, qT_sb), (wk_sb, bk_sb, kbias, kT_sb))
                ):
                    ppj = pp.tile([HD, T], F32, tag="pj")
                    nc.tensor.matmul(
                        ppj, lhsT=w_sb, rhs=xT_sb[:, i, :], start=True, stop=True
                    )
                    if has_b:
                        nc.tensor.matmul(
                            ppj, lhsT=b_sb, rhs=ones1, start=False, stop=True,
                            skip_group_check=True,
                        )
                    nc.vector.tensor_copy(dst, ppj)
                # v -> [t, hd] (token-major, the AV stationary operand)
                pv = pp.tile([128, TT, HD], F32, tag="pj")
                for j in range(TT):
                    nc.tensor.matmul(
                        pv[:, j, :],
                        lhsT=xT_sb[:, 2, j * 128 : (j + 1) * 128],
                        rhs=wv_sb, start=True, stop=True,
                    )
                vh_sb = pqkv.tile([128, TT, HD], BF16, tag="vh")
                nc.vector.tensor_copy(vh_sb, pv)
                qT_w[w], kT_w[w], vh_w[w] = qT_sb, kT_sb, vh_sb

            def stage_c(w):
                # dot head-pairs row-packed per kt (512-col spacing keeps
                # each head's output inside one PSUM bank), Exp per pair
                qT_sb, kT_sb = qT_w[w], kT_w[w]
                expT_sb = pexp.tile([128, TT, HEADS, T], BF16, tag="expT")
                for kt in range(TT):
                    for hp in range(HEADS // 2):
                        dt = psdot.tile([128, 2, 512], F32, tag="dot")
                        for hh in range(2):
                            h = 2 * hp + hh
                            nc.tensor.matmul(
                                dt[:, hh, 0:T],
                                lhsT=kT_sb[32 * h : 32 * (h + 1), kt * 128 : (kt + 1) * 128],
                                rhs=qT_sb[32 * h : 32 * (h + 1), :],
                                start=True, stop=True,
                                tile_position=(32 * h, 0),
                            )
                        nc.scalar.activation(
                            out=expT_sb[:, kt, 2 * hp : 2 * hp + 2, :],
                            in_=dt[:, :, 0:T],
                            func=mybir.ActivationFunctionType.Exp,
                        )
                expT_w[w] = expT_sb

            def stage_d(w):
                expT_sb, vh_sb = expT_w[w], vh_w[w]
                s_ps = pssav.tile([HD, T], F32, tag="s")
                av_ps = pssav.tile([HD, T], F32, tag="av")
                for kt in range(TT):
                    for h in range(HEADS):
                        nc.tensor.matmul(
                            s_ps[32 * h : 32 * (h + 1), :],
                            lhsT=ones32, rhs=expT_sb[:, kt, h, :],
                            start=(kt == 0), stop=(kt == TT - 1),
                            tile_position=(0, 32 * h), skip_group_check=True,
                        )
                        nc.tensor.matmul(
                            av_ps[32 * h : 32 * (h + 1), :],
                            lhsT=vh_sb[:, kt, 32 * h : 32 * (h + 1)],
                            rhs=expT_sb[:, kt, h, :],
                            start=(kt == 0), stop=(kt == TT - 1),
                            tile_position=(0, 32 * h), skip_group_check=True,
                        )
                rs_sb = prs.tile([HD, T], F32, tag="rs")
                nc.vector.reciprocal(out=rs_sb, in_=s_ps)
                aT_sb = pat.tile([HD, T], BF16, tag="aT")
                nc.vector.tensor_mul(aT_sb, av_ps, rs_sb)
                aT_w[w] = aT_sb

            def stage_e(w):
                aT_sb = aT_w[w]
                # out proj all 6 agents in one matmul; mean over agents as a
                # strided DVE reduce: q col c = k*96 + n*16 + r (u = 4r + k)
                z_ps = psz.tile([DIM, T], F32, tag="z")
                nc.tensor.matmul(
                    z_ps, lhsT=wp_sb, rhs=aT_sb, start=True, stop=True
                )
                zT_sb = ptail.tile([DIM, WTOK], F32, tag="zT")
                nc.vector.tensor_reduce(
                    out=zT_sb.rearrange("p (k r) -> p k r", k=KK),
                    in_=z_ps.rearrange("p (k n r) -> p k r n", k=KK, n=NAG),
                    axis=mybir.AxisListType.X,
                    op=mybir.AluOpType.add,
                )
                zt_ps = psz.tile([WTOK, DIM], F32, tag="zt")
                nc.tensor.transpose(out=zt_ps, in_=zT_sb, identity=identf[:, :])
                out_sb = ptail.tile([WTOK, DIM], F32, tag="osb")
                nc.vector.tensor_add(out_sb, zt_ps, skip_w[w])
                nc.gpsimd.dma_start(
                    out=outo[w].rearrange("(r k) d -> k r d", k=KK),
                    in_=out_sb,
                )

            for step in range(n_win + 3):
                if step < n_win:
                    stage_a(step)
                if 0 <= step - 1 < n_win:
                    stage_c(step - 1)
                if 0 <= step - 2 < n_win:
                    stage_d(step - 2)
                if 0 <= step - 3 < n_win:
                    stage_e(step - 3)

    return nc


def _split_multiwaits(nc, limit=1):
    """The staged walrus build rejects instructions carrying more than one
    sync-wait condition. Tile attaches several to some instructions (and the
    kernel-tail drain); peel the extras onto preceding engine NoOps. HW-only:
    CoreSim's sem bookkeeping rejects the injected NoOps."""
    for f in nc.m.functions:
        for bb in f.blocks:
            new_list = []
            for inst in bb.instructions:
                si = getattr(inst, "sync_info", None)
                waits = list(si.on_wait) if si is not None and si.on_wait else []
                if len(waits) > limit:
                    extra, keep = waits[:-limit], waits[-limit:]
                    for j in range(0, len(extra), limit):
                        nop = mybir.InstNoOp(
                            name=nc.get_next_instruction_name(),
                            engine=inst.engine,
                            ins=[],
                            outs=[],
                            sync_info=mybir.SyncInfo(
                                on_wait=extra[j : j + limit], on_update=[]
                            ),
                        )
                        new_list.append(nop)
                    si.on_wait = keep
                new_list.append(inst)
            if len(new_list) != len(bb.instructions):
                bb.instructions = new_list
    return nc


def _prep(inputs):
    """Host-side constant folding + window gather + shard. Returns
    (in_maps, qbias, kbias)."""
    f32 = np.float32
    q = np.asarray(inputs["q"], f32)
    k = np.asarray(inputs["k"], f32)
    v = np.asarray(inputs["v"], f32)
    skip = np.asarray(inputs["skip"], f32)
    gate = np.asarray(inputs["head_gate"], f32)
    lnqw, lnqb = np.asarray(inputs["ln_q_w"], f32), np.asarray(inputs["ln_q_b"], f32)
    lnkw, lnkb = np.asarray(inputs["ln_k_w"], f32), np.asarray(inputs["ln_k_b"], f32)
    lnvw, lnvb = np.asarray(inputs["ln_v_w"], f32), np.asarray(inputs["ln_v_b"], f32)
    wq, bq = np.asarray(inputs["wq"], f32), np.asarray(inputs["bq"], f32)
    wk, bk = np.asarray(inputs["wk"], f32), np.asarray(inputs["bk"], f32)
    wv, bv = np.asarray(inputs["wv"], f32), np.asarray(inputs["bv"], f32)
    wp, bp = np.asarray(inputs["wp"], f32), np.asarray(inputs["bp"], f32)

    # fold LN affine into the projections; fold softmax scale + head_gate
    # into the q side (dot*gate == (qh*gate).kh); fold sqrt(DIM) into all
    # three (device rstd_raw = rstd_true/sqrt(DIM))
    rdim = np.sqrt(np.float32(DIM))
    colscale = np.repeat(gate * SCALE, DHEAD)          # [HD]
    wq_f = (lnqw[:, None] * wq) * colscale[None, :] * rdim
    bq_f = lnqb @ wq * colscale + bq * colscale
    wk_f = lnkw[:, None] * wk * rdim
    bk_f = lnkb @ wk + bk
    wv_f = lnvw[:, None] * wv * rdim
    bv_f = lnvb @ wv + bv
    wp_f = wp / NAG
    # constant v offset passes straight through attention (softmax sums to 1)
    skip_c = bv_f @ wp + bp                             # [DIM]

    qbias = bool(np.any(bq_f != 0))
    kbias = bool(np.any(bk_f != 0))

    def windows(t):
        return t.transpose(0, 2, 3, 1, 4, 5, 6).reshape(NWIN, T, DIM)

    qkvw = np.ascontiguousarray(
        np.stack([windows(q), windows(k), windows(v)], axis=1)
    )  # [NWIN, 3, T, DIM]
    skw = (skip + skip_c).reshape(NWIN, WTOK, DIM)

    bf = ml_dtypes.bfloat16
    wq_b = np.ascontiguousarray(wq_f.astype(bf))
    wk_b = np.ascontiguousarray(wk_f.astype(bf))
    wv_b = np.ascontiguousarray(wv_f.astype(bf))
    wp_b = np.ascontiguousarray(wp_f.astype(bf))

    in_maps = []
    for c in range(N_CORES):
        sl = slice(c * WPC, (c + 1) * WPC)
        in_maps.append(
            {
                "qkvin": qkvw[sl],
                "skin": np.ascontiguousarray(skw[sl]),
                "wq": wq_b,
                "wk": wk_b,
                "wv": wv_b,
                "wp": wp_b,
                "bq": np.ascontiguousarray(bq_f[None, :]),
                "bk": np.ascontiguousarray(bk_f[None, :]),
            }
        )
    return in_maps, qbias, kbias


_BUILD_CACHE = {}


def _trace_available():
    try:
        from antenv.axon_hooks import get_axon_ntff_profile_hook  # noqa: F401

        return get_axon_ntff_profile_hook() is not None
    except Exception:
        return False


def run_sharded(in_maps, qbias, kbias, iters=1, trace=False):
    key = (qbias, kbias, iters)
    if key not in _BUILD_CACHE:
        # wait-splitting is for the walrus compiler only; CoreSim paths use
        # build_nc directly without it
        _BUILD_CACHE[key] = _split_multiwaits(
            build_nc(WPC, qbias=qbias, kbias=kbias, iters=iters)
        )
    nc = _BUILD_CACHE[key]
    return run_bass_kernel_spmd(
        nc, in_maps, core_ids=list(range(N_CORES)), trace=trace,
    )


def kernel(**inputs) -> np.ndarray:
    in_maps, qbias, kbias = _prep(inputs)
    trace = bool(int(os.environ.get("KERNEL_TRACE", "0"))) and _trace_available()
    res = run_sharded(in_maps, qbias, kbias, iters=1, trace=trace)
    if trace and res.exec_time_ns is not None:
        kernel.last_exec_time_ns = res.exec_time_ns
        kernel.last_trace = res.instructions_and_trace
    out = np.concatenate([r["out"] for r in res.results], axis=0)  # [128,64,128]
    out = out.reshape(B, X, Y, W1, W2, DIM)
    return np.ascontiguousarray(out.astype(np.float32))

